# revision 71
# baseline (speedup 1.0000x reference)
"""Trainium2 Bass kernel for nn_CoordinateGCN (8-layer GCN, tridiagonal adjacency).

Strategy (v2)
-------------
Pure data parallel over the batch: 64 items -> 8 NeuronCores x 8 items.
Feature-major activations x[d, n] resident in SBUF (1024 features on 8
partition chunks of 128, 600 nodes on the free axis).

Main matmuls run in fp8 (e4m3) with MatmulPerfMode.DoubleRow: each
instruction contracts 2x128 rows at 0.5 cycles/row (4x the bf16 rate).
Weights are host-scaled by 64 into fp8; the layer bias (also x64, fp8)
enters PSUM via a rank-1 DoubleRow matmul, so eviction is a single fused
DVE scalar_tensor_tensor per chunk:  z = (psum * 2^-6) + x.

LayerNorm per item-layer:
  mu:    ones(1/D) matmul partition-reduce of z -> psum row, ACT copy out
  t:     z - mu_b  (one big DVE 2x op, mu broadcast by PE ones-row matmul)
  var:   E[t^2]: ACT Square(t/4) -> fp8, then fp8 DoubleRow ones(1/64)
         reduce (scales combine to 1/1024)
  rstd:  single ACT Abs_reciprocal_sqrt(var + eps) row op
  apply: r = t*rstd_b (big DVE 2x), then per-e-tile ACT
         gelu(gamma*r + beta) writes the residual stream in place.

The adjacency aggregate (x_left + x + x_right) is built on GpSimd in two
big strided ops; the second fuses the bf16->fp8 downcast for the matmul
rhs.  Item-level software pipelining: LN stages of older items are
emitted between the e-tile matmul groups of the current item.
"""

import sys

sys.path.insert(0, "/opt/trn_rl_repo")

import numpy as np
import ml_dtypes

BF16 = ml_dtypes.bfloat16
FP8 = ml_dtypes.float8_e4m3

# Problem shapes (hardcoded per the harness contract).
B = 64
NCORES = 8
ITEMS = B // NCORES
P = 128
D = 1024  # input dim == embed dim
KD = D // P
E = 1024
KE = E // P
N = 600
NP = 604  # padded node columns; data at [2, 602), zeros elsewhere
COL0 = 2
L = 8
CH = 300
NCH = 2
SCH = (512, 88)  # stats chunking: each chunk stays inside one PSUM bank
LN_EPS = 1e-5
WSCALE = 64.0  # fp8 weight scale
IWSCALE = 1.0 / WSCALE
SQS = 0.25  # square input scale; SQS^2 * VRED_ONES = 1/D
VRED_ONES = 1.0 / 64.0
# e-tiles whose eviction runs on ACT (x-residual enters PSUM via a 64*I
# matmul, bias rides the ACT bias slot); the rest evict on DVE via
# scalar_tensor_tensor (bias enters PSUM via a rank-1 fp8 DR matmul).
EVICT_ACT = (1, 3, 5, 7)

_CACHE = {}


def _build_nc():
    from contextlib import ExitStack

    import concourse.bass as bass  # noqa: F401
    import concourse.tile as tile
    from concourse import bacc
    import concourse.mybir as mybir

    import bass_rust

    dt = mybir.dt
    F = mybir.ActivationFunctionType
    OP = mybir.AluOpType
    DR = mybir.MatmulPerfMode.DoubleRow
    VecPair = bass_rust.VecI64Pair

    nc = bacc.Bacc("TRN2", target_bir_lowering=False, debug=False, num_devices=NCORES)

    featT = nc.dram_tensor(
        "featT", [ITEMS, KD, P, N], dt.float8e4, kind="ExternalInput"
    ).ap()
    posb = nc.dram_tensor(
        "posb", [ITEMS, KE, P, N], dt.bfloat16, kind="ExternalInput"
    ).ap()
    # wts[0] = Wp (input projection), wts[1..L] = per-layer GCN weights, x64 fp8
    wts = nc.dram_tensor(
        "wts", [L + 1, KD, P, E], dt.float8e4, kind="ExternalInput"
    ).ap()
    # blv[l, 0, 0, :] = 64*bl, blv[l, 0, 1, :] = 0 (rank-1 DR bias lhsT rows)
    blv = nc.dram_tensor("blv", [L, 1, 2, E], dt.float8e4, kind="ExternalInput").ap()
    gam = nc.dram_tensor("gam", [L, P, KE], dt.float32, kind="ExternalInput").ap()
    bet = nc.dram_tensor("bet", [L, P, KE], dt.float32, kind="ExternalInput").ap()
    blf = nc.dram_tensor("blf", [L, P, KE], dt.float32, kind="ExternalInput").ap()
    id64 = nc.dram_tensor("id64", [P, P], dt.bfloat16, kind="ExternalInput").ap()
    wo = nc.dram_tensor("wo", [KD, P, 2], dt.bfloat16, kind="ExternalInput").ap()
    bo = nc.dram_tensor("bo", [2, 1], dt.float32, kind="ExternalInput").ap()
    outT = nc.dram_tensor("outT", [ITEMS, 2, N], dt.float32, kind="ExternalOutput").ap()

    with tile.TileContext(nc) as tc, ExitStack() as ctx:
        const = ctx.enter_context(tc.tile_pool(name="const", bufs=1))
        xpool = ctx.enter_context(tc.tile_pool(name="xres", bufs=1))
        wpool = ctx.enter_context(tc.tile_pool(name="wpool", bufs=2))
        lscal = ctx.enter_context(tc.tile_pool(name="lscal", bufs=2))
        aggp = ctx.enter_context(tc.tile_pool(name="aggp", bufs=2))
        agg8p = ctx.enter_context(tc.tile_pool(name="agg8p", bufs=3))
        zpool = ctx.enter_context(tc.tile_pool(name="zpool", bufs=4))
        sq8p = ctx.enter_context(tc.tile_pool(name="sq8p", bufs=2))
        bcp = ctx.enter_context(tc.tile_pool(name="bcp", bufs=4))
        smp = ctx.enter_context(tc.tile_pool(name="smp", bufs=4))
        obp = ctx.enter_context(tc.tile_pool(name="obp", bufs=2))
        pz = ctx.enter_context(tc.tile_pool(name="pz", bufs=3, space="PSUM"))
        pbc = ctx.enter_context(tc.tile_pool(name="pbc", bufs=2, space="PSUM"))
        pst = ctx.enter_context(tc.tile_pool(name="pst", bufs=3, space="PSUM"))

        # constants
        ones_col = const.tile([P, 1], dt.bfloat16)
        nc.vector.memset(ones_col[:], 1.0 / D)  # mu reduce: 1/D folded in
        ones_row = const.tile([1, P], dt.bfloat16)
        nc.vector.memset(ones_row[:], 1.0)
        ones2 = const.tile([1, 2, CH], dt.float8e4)
        nc.vector.memset(ones2[:], 1.0)  # rank-1 bias rhs (group1 hits 0-rows)
        vones = const.tile([P, 2, P], dt.float8e4)
        nc.vector.memset(vones[:], VRED_ONES)  # fp8 DR variance reduce lhsT
        eps_sb = const.tile([1, 1], dt.float32)
        nc.vector.memset(eps_sb[:], LN_EPS)
        bo_sb = const.tile([2, 1], dt.float32)
        nc.sync.dma_start(bo_sb[:], bo)
        id_sb = const.tile([P, P], dt.bfloat16)
        nc.sync.dma_start(id_sb[:], id64)
        wo_sb = const.tile([P, KD, 2], dt.bfloat16)
        nc.sync.dma_start(wo_sb[:], wo.rearrange("k p c -> p k c"))

        # Residual stream, resident for all 8 items: [P, item, d_chunk, node]
        x = xpool.tile([P, ITEMS, KD, NP], dt.bfloat16)
        nc.vector.memset(x[:], 0.0)

        # ---- software pipeline ----
        from collections import deque

        pending = deque()  # deque of (parity, per-item stage deque)
        pctr = [0]
        drain = [False]

        def point():
            # Half-rate, parity-phased pacing: each item advances every other
            # point; spreading the stage chain over ~2 slots smooths per-slot
            # engine load.  During the drain there is no matmul stream left to
            # pace against, so advance every item on every call.
            pctr[0] += 1
            for ent in list(pending):
                par, sl = ent
                if sl and (drain[0] or (pctr[0] + par) % 2 == 0):
                    sl.popleft()()
                if not sl:
                    pending.remove(ent)

        def head_chunk(it, c, ob):
            # coords.T = Wo.T @ x -> [2, 600] chunk, + bo
            cps = pz.tile([P, 512], dt.float32, tag="zps", name=f"cps_{it}_{c}")
            for k in range(KD):
                nc.tensor.matmul(
                    cps[0:2, 0:CH],
                    lhsT=wo_sb[:, k, :],
                    rhs=x[:, it, k, COL0 + c * CH : COL0 + (c + 1) * CH],
                    start=(k == 0),
                    stop=(k == KD - 1),
                )
            nc.scalar.activation(
                ob[:, c * CH : (c + 1) * CH],
                cps[0:2, 0:CH],
                F.Identity,
                bias=bo_sb[:, 0:1],
            )
            if c == NCH - 1:
                nc.sync.dma_start(outT[it], ob[:])

        def make_stages(it, z_sb, ga_sb, be_sb, last=False):
            st = {}

            def s1():  # mu partition-reduce (PE), z stays put
                tiles = []
                off = 0
                for w in SCH:
                    sp = pst.tile([P, 512], dt.float32, tag="st")
                    for k in range(KD):
                        nc.tensor.matmul(
                            sp[0:1, 0:w],
                            lhsT=ones_col[:, 0:1],
                            rhs=z_sb[:, k, off : off + w],
                            start=(k == 0),
                            stop=(k == KD - 1),
                        )
                    tiles.append(sp)
                    off += w
                st["mu_ps"] = tiles

            def s2():  # mu row -> sbuf, broadcast, mu_b -> sbuf
                mu_sb = smp.tile([1, N], dt.bfloat16, tag="mu")
                off = 0
                for sp, w in zip(st["mu_ps"], SCH):
                    nc.scalar.copy(mu_sb[0:1, off : off + w], sp[0:1, 0:w])
                    off += w
                mu_b = bcp.tile([P, N], dt.bfloat16, tag="mub")
                off = 0
                for ci, w in enumerate(SCH):
                    bp = pbc.tile([P, 512], dt.float32, tag="bc", name=f"mb{it}_{ci}")
                    nc.tensor.matmul(
                        bp[:, 0:w],
                        lhsT=ones_row[0:1, :],
                        rhs=mu_sb[0:1, off : off + w],
                        start=True,
                        stop=True,
                    )
                    nc.scalar.copy(mu_b[:, off : off + w], bp[:, 0:w])
                    off += w
                st["mu_b"] = mu_b

            def s3():  # t = z - mu_b  (in place, one big DVE 2x op)
                nc.vector.tensor_tensor(
                    z_sb[:],
                    z_sb[:],
                    st["mu_b"][:, None, :].to_broadcast((P, KD, N)),
                    op=OP.subtract,
                )

            def s4():  # tsq: k-chunks 0-3 on ACT, 4-7 on DVE, both -> fp8/16
                # ACT half: (t*SQS)^2 in fp8; DVE half: t*t unscaled -> bf16
                # is 2x but forces a slow reduce; fp8 out (1x) keeps DR reduce.
                SQA = 3  # k-chunks squared on ACT; the rest on DVE
                tsq = sq8p.tile([P, KD, N], dt.float8e4, tag="tsq")
                nc.scalar.activation(
                    tsq[:, 0:SQA, :], z_sb[:, 0:SQA, :], F.Square, scale=SQS
                )
                nc.vector.scalar_tensor_tensor(
                    tsq[:, SQA:, :],
                    z_sb[:, SQA:, :],
                    SQS * SQS,
                    z_sb[:, SQA:, :],
                    op0=OP.mult,
                    op1=OP.mult,
                )
                st["tsq"] = tsq

            def s5():  # var reduce: fp8 DR over tsq + bf16 plain over tsqb
                tiles = []
                off = 0
                for ci, w in enumerate(SCH):
                    vp = pst.tile([P, 512], dt.float32, tag="st", name=f"v{it}_{ci}")
                    for kp in range(KD // 2):
                        nc.tensor.matmul(
                            vp[:, 0:w],
                            lhsT=vones[:],
                            rhs=st["tsq"][:, 2 * kp : 2 * kp + 2, off : off + w],
                            start=(kp == 0),
                            stop=(kp == KD // 2 - 1),
                            perf_mode=DR,
                        )
                    tiles.append(vp)
                    off += w
                st["v_ps"] = tiles

            def s6():  # rstd row + broadcast
                rstd_sb = smp.tile([1, N], dt.bfloat16, tag="rstd")
                off = 0
                for vp, w in zip(st["v_ps"], SCH):
                    nc.scalar.activation(
                        rstd_sb[0:1, off : off + w],
                        vp[0:1, 0:w],
                        F.Abs_reciprocal_sqrt,
                        bias=eps_sb[0:1, 0:1],
                    )
                    off += w
                rstd_b = bcp.tile([P, N], dt.bfloat16, tag="rsb")
                off = 0
                for ci, w in enumerate(SCH):
                    bp = pbc.tile([P, 512], dt.float32, tag="bc", name=f"rb{it}_{ci}")
                    nc.tensor.matmul(
                        bp[:, 0:w],
                        lhsT=ones_row[0:1, :],
                        rhs=rstd_sb[0:1, off : off + w],
                        start=True,
                        stop=True,
                    )
                    nc.scalar.copy(rstd_b[:, off : off + w], bp[:, 0:w])
                    off += w
                st["rstd_b"] = rstd_b

            def s7():  # r = t * rstd_b (in place, big DVE 2x)
                nc.vector.tensor_tensor(
                    z_sb[:],
                    z_sb[:],
                    st["rstd_b"][:, None, :].to_broadcast((P, KD, N)),
                    op=OP.mult,
                )

            def s8():  # gelu(gamma*r + beta) -> x (8 ACT ops)
                for ke in range(KE):
                    nc.scalar.activation(
                        x[:, it, ke, COL0 : COL0 + N],
                        z_sb[:, ke, :],
                        F.Gelu,
                        bias=be_sb[:, ke : ke + 1],
                        scale=ga_sb[:, ke : ke + 1],
                    )

            stages = [s1, s2, s3, s4, s5, s6, s7, s8]
            if last:
                def s9():
                    ob = obp.tile([2, N], dt.float32, tag="ob", name=f"ob_{it}")
                    for c in range(NCH):
                        head_chunk(it, c, ob)

                stages.append(s9)
            return deque(stages)

        w_tiles = {}

        def load_w(l):
            w_tiles[l] = wpool.tile([P, KD, E], dt.float8e4, tag="w", name=f"w_{l}")
            nc.sync.dma_start(w_tiles[l][:], wts[l].rearrange("k p e -> p k e"))

        def emit_agg(l, it, slot):
            """rhs for (l, it): fp8 adjacency aggregate (GCN) or DMA'd fp8
            features (input projection).  Called one slot ahead."""
            if l > 0:
                aggA = aggp.tile([P, KD, N], dt.bfloat16, tag="aggA", name=f"aA_{slot}")
                agg8 = agg8p.tile([P, KD, N], dt.float8e4, tag="agg8", name=f"a8_{slot}")
                # x_left + x_right on GpSimd (its only bulk job)
                nc.gpsimd.tensor_tensor(
                    aggA[:],
                    x[:, it, :, COL0 - 1 : COL0 - 1 + N],
                    x[:, it, :, COL0 + 1 : COL0 + 1 + N],
                    op=OP.add,
                )
                # + center with fp8 downcast: half DVE (1x), half GpSimd
                h = KD // 2
                nc.vector.tensor_tensor(
                    agg8[:, 0:h, :],
                    aggA[:, 0:h, :],
                    x[:, it, 0:h, COL0 : COL0 + N],
                    op=OP.add,
                )
                nc.gpsimd.tensor_tensor(
                    agg8[:, h:, :],
                    aggA[:, h:, :],
                    x[:, it, h:, COL0 : COL0 + N],
                    op=OP.add,
                )
                return agg8, None
            agg8 = agg8p.tile([P, KD, N], dt.float8e4, tag="agg8", name=f"a8_{slot}")
            nc.gpsimd.dma_start(agg8[:], featT[it].rearrange("k p n -> p k n"))
            pb_sb = zpool.tile([P, KD, N], dt.bfloat16, tag="z", name=f"pb_{slot}")
            nc.gpsimd.dma_start(pb_sb[:], posb[it].rearrange("k p n -> p k n"))
            return agg8, pb_sb

        plan = [(l, it) for l in range(L + 1) for it in range(ITEMS)]
        load_w(0)
        layer_params = {}
        AHEAD = 1  # rhs (agg / feat-DMA) lookahead in slots; in-order
        # engine queues make deeper lookahead counterproductive (prefetch
        # ops delay the current slot's critical-path work)
        agg_q = deque(emit_agg(*plan[k], k) for k in range(AHEAD))

        for j, (l, it) in enumerate(plan):
            if it == 0 and l > 0 and l not in layer_params:
                bl_sb = lscal.tile([1, 2, E], dt.float8e4, tag="bl", name=f"bl_{l}")
                nc.sync.dma_start(bl_sb[:], blv[l - 1])
                ga_sb = lscal.tile([P, KE], dt.float32, tag="ga", name=f"ga_{l}")
                nc.sync.dma_start(ga_sb[:], gam[l - 1])
                be_sb = lscal.tile([P, KE], dt.float32, tag="be", name=f"be_{l}")
                nc.sync.dma_start(be_sb[:], bet[l - 1])
                bf_sb = lscal.tile([P, KE], dt.float32, tag="bf", name=f"bf_{l}")
                nc.sync.dma_start(bf_sb[:], blf[l - 1])
                layer_params[l] = (bl_sb, ga_sb, be_sb, bf_sb)
            if l > 0:
                bl_sb, ga_sb, be_sb, bf_sb = layer_params[l]
            if it == 0:
                w_sb = w_tiles.pop(l)
            if it == 2 and l < L:
                load_w(l + 1)  # prefetch next layer's weights mid-layer

            agg8, pb_sb = agg_q.popleft()
            if j + AHEAD < len(plan):
                agg_q.append(emit_agg(*plan[j + AHEAD], j + AHEAD))

            if l > 0:
                z_sb = zpool.tile([P, KD, N], dt.bfloat16, tag="z", name=f"z_{j}")

            for ke in range(KE):
                act_path = l > 0 and ke in EVICT_ACT
                for c in range(NCH):
                    zps = pz.tile(
                        [P, 512], dt.float32, tag="zps", name=f"zps_{j}_{ke}_{c}"
                    )
                    xsl = x[:, it, ke, COL0 + c * CH : COL0 + (c + 1) * CH]
                    for kp in range(KD // 2):
                        nc.tensor.matmul(
                            zps[:, 0:CH],
                            lhsT=w_sb[:, 2 * kp : 2 * kp + 2, ke * P : (ke + 1) * P],
                            rhs=agg8[:, 2 * kp : 2 * kp + 2, c * CH : (c + 1) * CH],
                            start=(kp == 0),
                            stop=(l == 0 and kp == KD // 2 - 1),
                            perf_mode=DR,
                        )
                    if act_path:
                        # residual via 64*I matmul; bias rides the ACT bias slot
                        nc.tensor.matmul(
                            zps[:, 0:CH],
                            lhsT=id_sb[:],
                            rhs=xsl,
                            start=False,
                            stop=True,
                        )
                        nc.scalar.activation(
                            z_sb[:, ke, c * CH : (c + 1) * CH],
                            zps[:, 0:CH],
                            F.Identity,
                            scale=IWSCALE,
                            bias=bf_sb[:, ke : ke + 1],
                        )
                        continue
                    if l > 0:
                        # += 64*bl[e] via rank-1 fp8 DoubleRow (group1 rows are 0)
                        nc.tensor.matmul(
                            zps[:, 0:CH],
                            lhsT=bl_sb[:, :, ke * P : (ke + 1) * P],
                            rhs=ones2[:],
                            start=False,
                            stop=True,
                            perf_mode=DR,
                        )
                    dst = (
                        x[:, it, ke, COL0 + c * CH : COL0 + (c + 1) * CH]
                        if l == 0
                        else z_sb[:, ke, c * CH : (c + 1) * CH]
                    )
                    other = pb_sb[:, ke, c * CH : (c + 1) * CH] if l == 0 else xsl
                    # z = psum/64 + residual (fused evict)
                    nc.vector.scalar_tensor_tensor(
                        dst, zps[:, 0:CH], IWSCALE, other, op0=OP.mult, op1=OP.add
                    )
                point()

            if l > 0:
                pending.append(
                    (it % 2, make_stages(it, z_sb, ga_sb, be_sb, last=(l == L)))
                )

        drain[0] = True
        while pending:
            point()

    nc.compile()
    return nc


def _get_nc():
    if "nc" not in _CACHE:
        _CACHE["nc"] = _build_nc()
    return _CACHE["nc"]


def _prep_inputs(features, positions, Wp, bp, pos_tab, Wl, bl, gamma, beta, Wo, bo):
    """Host-side packing: transpose/cast to the device layouts."""
    features = np.ascontiguousarray(np.asarray(features, np.float32))
    positions = np.asarray(positions)
    Wp = np.asarray(Wp, np.float32)
    bp = np.asarray(bp, np.float32)
    pos_tab = np.asarray(pos_tab, np.float32)
    Wl = np.asarray(Wl, np.float32)
    bl = np.asarray(bl, np.float32)
    gamma = np.asarray(gamma, np.float32)
    beta = np.asarray(beta, np.float32)
    Wo = np.asarray(Wo, np.float32)
    bo = np.asarray(bo, np.float32)

    featT = (
        features.transpose(0, 2, 1).reshape(B, KD, P, N).astype(FP8)
    )  # [B, k, p, n]
    # bp + pos_tab[positions]: [B, n, e] -> transposed/bf16 per item
    pe = pos_tab[positions] + bp[None, None, :]
    posbT = pe.transpose(0, 2, 1).reshape(B, KE, P, N).astype(BF16)

    wts = np.concatenate([Wp[None], Wl], axis=0)  # [L+1, d, e]
    wts = (wts * WSCALE).reshape(L + 1, KD, P, E).astype(FP8)
    blv = np.zeros((L, 1, 2, E), np.float32)
    blv[:, 0, 0, :] = bl * WSCALE
    blv = blv.astype(FP8)
    gam = np.ascontiguousarray(gamma.reshape(L, KE, P).transpose(0, 2, 1))  # [L, P, KE]
    bet = np.ascontiguousarray(beta.reshape(L, KE, P).transpose(0, 2, 1))
    blf = np.ascontiguousarray(bl.reshape(L, KE, P).transpose(0, 2, 1)).astype(
        np.float32
    )
    id64 = (np.eye(P, dtype=np.float32) * WSCALE).astype(BF16)
    woT = Wo.reshape(KD, P, 2).astype(BF16)
    bov = bo.reshape(2, 1)

    in_maps = []
    for c in range(NCORES):
        sl = slice(c * ITEMS, (c + 1) * ITEMS)
        in_maps.append(
            {
                "featT": np.ascontiguousarray(featT[sl]),
                "posb": np.ascontiguousarray(posbT[sl]),
                "wts": wts,
                "blv": blv,
                "gam": gam,
                "bet": bet,
                "blf": blf,
                "id64": id64,
                "wo": woT,
                "bo": bov,
            }
        )
    return in_maps


def run_device(in_maps, trace=False, **kwargs):
    """Compile (cached) and run the SPMD kernel; returns BassKernelResults."""
    from concourse import bass_utils

    nc = _get_nc()
    res = bass_utils.run_bass_kernel_spmd(
        nc, in_maps, core_ids=list(range(NCORES)), trace=trace, **kwargs
    )
    return res


def kernel(**inputs) -> np.ndarray:
    in_maps = _prep_inputs(
        inputs["features"],
        inputs["positions"],
        inputs["Wp"],
        inputs["bp"],
        inputs["pos_tab"],
        inputs["Wl"],
        inputs["bl"],
        inputs["gamma"],
        inputs["beta"],
        inputs["Wo"],
        inputs["bo"],
    )
    res = run_device(in_maps, trace=False)
    out = np.empty((B, 600, 2), np.float32)
    for c in range(NCORES):
        o = res.results[c]["outT"]  # [ITEMS, 2, N]
        out[c * ITEMS : (c + 1) * ITEMS] = o.transpose(0, 2, 1)
    out[:, 0, :] = [0.0, 0.0]
    out[:, -1, :] = [600.0, 0.0]
    return out
